# revision 25
# baseline (speedup 1.0000x reference)
"""Trainium2 Bass kernel for nn_Matposer_51007031608225.

Only the diagonal of the reference's [512,300,300] bmm is needed:

    diagT[k, e] = sum_l a_k[l] * (scale*emb1[x1[k,l],e] + pe[l,e])
    a_k[l]      = scale*emb2[x2[k,l],k] + pe[l,k]

Phase 1 (SPMD x8, k-sharded 38 per core, 5 pipelined chunks of 8 k):
  - emb1 rows gathered as fp16 (elem 384 = 768B, the 256B-granule optimum):
    halves the dominant DMA stream vs fp32.
  - the emb2 a-values come from an SBUF-resident per-core column slice
    (sequential fp16 load, [128 partitions = 16 vocab-chunks x 8 groups])
    via gpsimd.ap_gather: call c group g extracts k=c*8+g's 512 values as
    16 per-partition candidates; a host one-hot mask + a one-matmul
    group-sum reduce picks the right vocab chunk/parity.  This replaces
    the per-pair 256B slab dma_gather (27.7us of descriptor-bound DMA)
    with ~14us of otherwise-idle GPSIMD time.
  - a-vectors are transposed (PE identity-matmul) into the wrapped [128,4]
    lhsT layout matching the emb1 gather's row placement; the pe-term
    matmuls accumulate into the same PSUM rows as the per-k matvecs, so a
    single combined diag slice is stored.
Phase 2 (tiny, e-sharded x8): after the host concatenates/re-slices the
  per-core diag rows, relu(diag @ w1.T + b1) @ w2.T + b2 and softmax.
"""

import numpy as np
from contextlib import ExitStack

import concourse.bass as bass
import concourse.bacc as bacc
import concourse.tile as tile
import concourse.mybir as mybir
from concourse import library_config
from concourse.bass_utils import run_bass_kernel_spmd

F32 = mybir.dt.float32
F16 = mybir.dt.float16
I16 = mybir.dt.int16
U8 = mybir.dt.uint8

D = 300          # d_model
L = 512          # sequence length
V = 32000        # vocab
OUT = 4
NCORES = 8
NK = 38          # k's per core (8*38 = 304 >= 300)
EP1 = 384        # padded emb1 row in fp16 (768B = 3x256B)
NCALL = 5
# (base, width) k-chunks; the last chunk's rows are gathered j-major in four
# sub-gathers so its matvec trails the final DMA quarters (short serial tail)
CHUNKS = [(0, 8), (8, 8), (16, 8), (24, 8), (32, 6)]
NCH = len(CHUNKS)
JSPLIT = 1
VC = V // 16     # vocab entries per partition in the ap_gather table (2000)
NBLK = VC // 2   # d=2 blocks per partition (1000)
SCALE = float(np.sqrt(np.float32(D)))


# ---------------------------------------------------------------- phase 1

def _build_phase1(skip=()):
    nc = bacc.Bacc("TRN2", target_bir_lowering=False, debug=False,
                   num_devices=NCORES, num_swdge_queues=2)

    emb1f_d = nc.dram_tensor("emb1f", [V, EP1], F16, kind="ExternalInput").ap()
    x1w_d = nc.dram_tensor("x1w", [128, NK * 32], I16, kind="ExternalInput").ap()
    tab_d = nc.dram_tensor("tab", [128, NCALL * VC], F16, kind="ExternalInput").ap()
    aidx_d = nc.dram_tensor("aidx", [128, NCALL * 32], I16, kind="ExternalInput").ap()
    msk_d = nc.dram_tensor("msk", [128, NCALL * 2 * L], U8, kind="ExternalInput").ap()
    g8_d = nc.dram_tensor("g8", [128, 8], F16, kind="ExternalInput").ap()
    pe4_d = nc.dram_tensor("pe4", [128, 4 * D], F16, kind="ExternalInput").ap()
    pec_d = nc.dram_tensor("pec", [128, NCALL * 32], F16, kind="ExternalInput").ap()
    id8_d = nc.dram_tensor("id8", [8, 8], F32, kind="ExternalInput").ap()
    diag_d = nc.dram_tensor("diag", [NCH * 8, D], F32, kind="ExternalOutput").ap()

    with tile.TileContext(nc) as tc, ExitStack() as ctx:
        nc.gpsimd.load_library(library_config.ap_gather)
        cpool = ctx.enter_context(tc.tile_pool(name="consts", bufs=1))
        g1pool = ctx.enter_context(tc.tile_pool(name="g1", bufs=1))
        spool = ctx.enter_context(tc.tile_pool(name="small", bufs=1))
        appool = ctx.enter_context(tc.tile_pool(name="apg", bufs=2))
        afpool = ctx.enter_context(tc.tile_pool(name="af", bufs=1))
        pk_ps = ctx.enter_context(tc.tile_pool(name="pk", bufs=2, space="PSUM"))
        gs_ps = ctx.enter_context(tc.tile_pool(name="gs", bufs=2, space="PSUM"))
        tp_ps = ctx.enter_context(tc.tile_pool(name="tp", bufs=2, space="PSUM"))

        # x1w first (gates the gather desc-gen), then ap_gather deps
        x1w = cpool.tile([128, NK * 32], I16)
        nc.sync.dma_start(x1w[:], x1w_d[:])
        tab = cpool.tile([128, NCALL * VC], F16)
        nc.sync.dma_start(tab[:], tab_d[:])
        aidx = cpool.tile([128, NCALL * 32], I16)
        nc.sync.dma_start(aidx[:], aidx_d[:])
        msk = cpool.tile([128, NCALL * 2 * L], U8)
        nc.sync.dma_start(msk[:], msk_d[:])
        g8 = cpool.tile([128, 8], F16)
        nc.sync.dma_start(g8[:], g8_d[:])
        id8 = cpool.tile([8, 8], F32)
        nc.sync.dma_start(id8[:], id8_d[:])
        pe4 = cpool.tile([128, 4 * D], F16)
        nc.sync.dma_start(pe4[:], pe4_d[:])
        pec = cpool.tile([128, NCALL * 32], F16)
        nc.sync.dma_start(pec[:], pec_d[:])

        tabv = tab[:].rearrange("p (c b d) -> p c b d", c=NCALL, d=2)
        stg = spool.tile([8, NCH * D], F32)     # staged diag rows, stored once

        # ---- loop A: all emb1 gathers issued first (desc-gen up front, one
        # buffer per chunk so transfers stream back-to-back on the DMA device)
        g1s = []
        for ci, (base, chw) in enumerate(CHUNKS):
            last = (ci == NCH - 1) and JSPLIT > 1
            g1 = g1pool.tile([128, chw * 4 * EP1], F16, tag=f"g1_{ci}")
            if not last:
                ni = chw * L
                nc.gpsimd.dma_gather(
                    out_ap=g1[:].rearrange("p (c e) -> p c e", e=EP1),
                    in_ap=emb1f_d[:],
                    idxs_ap=x1w[:, base * 32:(base + chw) * 32],
                    num_idxs=ni,
                    num_idxs_reg=ni,
                    elem_size=EP1,
                    single_packet=False,
                    queue_num=ci % 2,
                )
            else:
                ni = chw * 128
                for j in range(JSPLIT):
                    nc.gpsimd.dma_gather(
                        out_ap=g1[:, j * chw * EP1:(j + 1) * chw * EP1]
                            .rearrange("p (c e) -> p c e", e=EP1),
                        in_ap=emb1f_d[:],
                        idxs_ap=x1w[:, base * 32 + j * chw * 8:
                                    base * 32 + (j + 1) * chw * 8],
                        num_idxs=ni,
                        num_idxs_reg=ni,
                        elem_size=EP1,
                        single_packet=False,
                        queue_num=j % 2,
                    )
            g1s.append(g1)

        # ---- loop B: a-value chains for every call (independent of emb1)
        affs, sws = [], []
        for c in range(NCALL):
            raw = appool.tile([128, 2 * L], F16, tag="raw")
            nc.gpsimd.ap_gather(
                out_ap=raw[:].rearrange("p (i d) -> p i d", d=2),
                in_ap=tabv[:, c],
                idxs_ap=aidx[:, c * 32:(c + 1) * 32],
                channels=128, num_elems=NBLK, d=2, num_idxs=L,
            )
            masked = appool.tile([128, 2 * L], F16, tag="mskd")
            nc.vector.memset(masked[:], 0.0)
            nc.vector.copy_predicated(
                masked[:], msk[:, c * 2 * L:(c + 1) * 2 * L], raw[:])
            asb = appool.tile([8, L], F32, tag="asb")
            for h in range(2):
                ps = gs_ps.tile([8, L], F32, tag="gs")
                nc.tensor.matmul(out=ps[:], lhsT=g8[:],
                                 rhs=masked[:, h * L:(h + 1) * L],
                                 start=True, stop=True)
                psv = ps[:].rearrange("g (i d) -> g i d", d=2)
                HL = L // 2
                tmp = appool.tile([8, HL], F32, tag=f"tmp{h}")
                nc.vector.tensor_copy(tmp[:], psv[:, :, 0])
                nc.vector.tensor_tensor(
                    out=asb[:, h * HL:(h + 1) * HL],
                    in0=tmp[:], in1=psv[:, :, 1],
                    op=mybir.AluOpType.add)
            # transpose [8, 512] -> wrapped [128, (j g)] fp16
            a_full = afpool.tile([128, 32], F16, tag=f"af{c}")
            s_wide = afpool.tile([128, 4 * 64], F16, tag=f"sw{c}")
            swv = s_wide[:].rearrange("p (j g m) -> p j g m", j=4, g=8)
            nc.vector.memset(s_wide[:], 0.0)
            for j in range(4):
                tp = tp_ps.tile([128, 8], F32, tag="tp")
                nc.tensor.transpose(tp[:], asb[:, j * 128:(j + 1) * 128], id8[:])
                nc.scalar.activation(
                    a_full[:, j * 8:j * 8 + 8], tp[:],
                    mybir.ActivationFunctionType.Copy, scale=SCALE)
            nc.vector.tensor_tensor(
                out=a_full[:], in0=a_full[:],
                in1=pec[:, c * 32:(c + 1) * 32],
                op=mybir.AluOpType.add)
            afv = a_full[:].rearrange("p (j g) -> p j g", j=4)
            for j in range(4):
                # scatter scale*a into the block-diagonal lhsT slots
                nc.vector.tensor_scalar_mul(
                    swv[:, j].rearrange("p g m -> p (g m)")[:, ::9],
                    afv[:, j], SCALE)
            affs.append(afv)
            sws.append(swv)

        # ---- loop C: pe-term + per-k matvecs, one PSUM tile per chunk
        for ci, (base, chw) in enumerate(CHUNKS):
            c, g0 = base // 8, base % 8
            afv, swv, g1 = affs[c], sws[c], g1s[ci]
            pk = pk_ps.tile([8, D], F32, tag="pk")
            for j in range(4):
                nc.tensor.matmul(
                    out=pk[0:chw, :],
                    lhsT=afv[:, j, g0:g0 + chw],
                    rhs=pe4[:, j * D:(j + 1) * D],
                    start=(j == 0), stop=False,
                    skip_group_check=True,
                )
            last = (ci == NCH - 1) and JSPLIT > 1
            order = ([(j, gg) for j in range(4) for gg in range(chw)]
                     if last else
                     [(j, gg) for gg in range(chw) for j in range(4)])
            for n, (j, gg) in enumerate(order):
                blk = (j * chw + gg) if last else (gg * 4 + j)
                nc.tensor.matmul(
                    out=pk[0:chw, :],
                    lhsT=swv[:, j, g0 + gg, g0:g0 + chw],
                    rhs=g1[:, blk * EP1:blk * EP1 + D],
                    start=False, stop=(n == len(order) - 1),
                    skip_group_check=True,
                )
            nc.scalar.activation(stg[0:chw, ci * D:(ci + 1) * D], pk[0:chw, :],
                                 mybir.ActivationFunctionType.Copy, scale=1.0)

        nc.sync.dma_start(
            diag_d[:].rearrange("(c g) e -> g c e", c=NCH),
            stg[:].rearrange("g (c e) -> g c e", c=NCH))

    nc.compile()
    return nc


EC = 38   # e-columns of the head computed per core (8*38 = 304 >= 300)
DP = 384


def _build_phase2s():
    """e-sharded head: every core gets the full diag k-rows but only its own
    38-column e-slice; computes [38, 4] output rows.  All inputs arrive in
    one packed [DP, 343] f32 tensor: [w1T | dS | w2T+b2 | b1]; b2 rides as
    w2T's row 300 against a ones-row injected into hT, and the softmax skips
    the max-subtraction (logits are O(50) at most, safe in f32 exp)."""
    PW = D + 1 + EC + OUT + 1    # 344 packed columns (w1T gets a zero col)
    nc = bacc.Bacc("TRN2", target_bir_lowering=False, debug=False,
                   num_devices=NCORES)

    pk_d = nc.dram_tensor("pk2", [DP, PW], F32, kind="ExternalInput").ap()
    out_d = nc.dram_tensor("out", [EC, OUT], F32, kind="ExternalOutput").ap()

    with tile.TileContext(nc) as tc, ExitStack() as ctx:
        pool = ctx.enter_context(tc.tile_pool(name="p2", bufs=1))
        psum = ctx.enter_context(tc.tile_pool(name="ps2", bufs=1, space="PSUM"))

        pkt = pool.tile([128, 3 * PW], F32)
        pkv = pk_d[:].rearrange("(c p) x -> c p x", p=128)
        for i in range(3):
            nc.sync.dma_start(pkt[:, i * PW:(i + 1) * PW], pkv[i])
        w1T = [pkt[:, i * PW:i * PW + D + 1] for i in range(3)]
        dT = [pkt[:, i * PW + D + 1:i * PW + D + 1 + EC] for i in range(3)]
        w2t = [pkt[:, i * PW + D + 1 + EC:i * PW + D + 1 + EC + OUT]
               for i in range(3)]
        b1t = [pkt[:, i * PW + PW - 1:i * PW + PW] for i in range(3)]

        # hT[j, e'] = relu(sum_k w1T[k, j] dT[k, e'] + b1[j]); j=300 is an
        # all-ones row (w1T col 300 = 0, b1[300] = 1) pairing with w2T row
        # 300 = b2, so the b2 bias rides the logits matmul
        JC = [(0, 128), (128, 128), (256, 45)]
        hT = []
        for jm, (j0, jn) in enumerate(JC):
            ph = psum.tile([128, EC], F32, tag=f"ph{jm}", space="PSUM")
            for kc in range(3):
                nc.tensor.matmul(
                    out=ph[:jn, :],
                    lhsT=w1T[kc][:, j0:j0 + jn],
                    rhs=dT[kc],
                    start=(kc == 0), stop=(kc == 2))
            th = pool.tile([128, EC], F32, tag=f"h{jm}")
            nc.scalar.activation(th[:jn, :], ph[:jn, :],
                                 mybir.ActivationFunctionType.Relu,
                                 bias=b1t[jm][:jn, :], scale=1.0)
            hT.append(th)

        # logits[e', o] = sum_j hT[j, e'] w2T[j, o]  (+b2 via ones-row)
        pl = psum.tile([128, OUT], F32, tag="pl", space="PSUM")
        for jm, (j0, jn) in enumerate(JC):
            nc.tensor.matmul(
                out=pl[:EC, :],
                lhsT=hT[jm][:jn, :],
                rhs=w2t[jm][:jn, :],
                start=(jm == 0), stop=(jm == 2))
        nmax = pool.tile([128, 1], F32, tag="nm")
        nc.vector.reduce_max(nmax[:EC, :], pl[:EC, :],
                             axis=mybir.AxisListType.X, negate=True)
        ex = pool.tile([128, OUT], F32, tag="ex")
        ssum = pool.tile([128, 1], F32, tag="ss")
        nc.scalar.activation(ex[:EC, :], pl[:EC, :],
                             mybir.ActivationFunctionType.Exp,
                             bias=nmax[:EC, :], scale=1.0,
                             accum_out=ssum[:EC, :])
        rcp = pool.tile([128, 1], F32, tag="rc")
        nc.vector.reciprocal(rcp[:EC, :], ssum[:EC, :])
        so = pool.tile([128, OUT], F32, tag="so")
        nc.vector.tensor_scalar_mul(so[:EC, :], ex[:EC, :], rcp[:EC, :])
        nc.sync.dma_start(out_d[:], so[:EC, :])

    nc.compile()
    return nc


_CACHE = {}
FUSED = False   # kept for test.py compatibility


def _phase1(fused=False):
    if "p1" not in _CACHE:
        _CACHE["p1"] = _build_phase1()
    return _CACHE["p1"]


def _phase2s():
    if "p2s" not in _CACHE:
        _CACHE["p2s"] = _build_phase2s()
    return _CACHE["p2s"]


# ---------------------------------------------------------------- host glue

def _pe_table():
    pos = np.arange(L, dtype=np.float32)[:, None]
    div = np.exp(np.arange(0, D, 2, dtype=np.float32)
                 * np.float32(-np.log(10000.0) / D))
    pe = np.zeros((L, D), dtype=np.float32)
    pe[:, 0::2] = np.sin(pos * div)
    pe[:, 1::2] = np.cos(pos * div)
    return pe


def _wrap_idx(rows):
    """rows [nk, 512] -> int16 [128, nk*32] in dma_gather's wrapped layout
    (per CHUNKS blocks; idx i of a chunk sits at [i%16, blockcol+i//16],
    replicated down all 128 partitions)."""
    out = np.zeros((16, rows.shape[0] * 32), dtype=np.int16)
    for ci, (base, chw) in enumerate(CHUNKS):
        blk = rows[base:base + chw]                     # [chw, 512]
        if ci == len(CHUNKS) - 1 and JSPLIT > 1:
            # j-major stream: position = j*chw*128 + k*128 + r
            seq = blk.reshape(chw, 4, 128).transpose(1, 0, 2).reshape(-1)
        else:
            seq = blk.reshape(-1)                       # chw*512, l-major
        out[:, base * 32:base * 32 + chw * 32] = seq.reshape(-1, 16).T
    return np.tile(out, (8, 1))


def kernel(x1, x2, emb1, emb2, w1, b1, w2, b2, _trace=(False, False)):
    x1 = np.asarray(x1); x2 = np.asarray(x2)
    emb1 = np.asarray(emb1, dtype=np.float32)
    emb2 = np.asarray(emb2, dtype=np.float32)
    w1 = np.asarray(w1, dtype=np.float32); b1 = np.asarray(b1, dtype=np.float32)
    w2 = np.asarray(w2, dtype=np.float32); b2 = np.asarray(b2, dtype=np.float32)

    pe = _pe_table()
    emb1f = np.zeros((V, EP1), dtype=np.float16)
    emb1f[:, :D] = emb1.astype(np.float16)

    # pe4: [p, j*300+e] = pe[j*128+p, e]
    pe4 = np.ascontiguousarray(
        pe.reshape(4, 128, D).transpose(1, 0, 2).reshape(128, 4 * D)
    ).astype(np.float16)

    g8 = np.zeros((128, 8), dtype=np.float16)
    for g in range(8):
        g8[16 * g:16 * (g + 1), g] = 1.0
    id8 = np.eye(8, dtype=np.float32)

    in_maps = []
    for core in range(NCORES):
        k0 = NK * core
        x1w = _wrap_idx(x1[k0:k0 + NK].astype(np.int64))

        x2c = x2[k0:k0 + NK].astype(np.int64)            # [38, 512]
        tabsb = np.zeros((128, NCALL, NBLK, 2), dtype=np.float16)
        aidx = np.zeros((128, NCALL, 32), dtype=np.int16)
        mskw = np.zeros((128, NCALL, L, 2), dtype=np.uint8)
        pec = np.zeros((128, NCALL, 4, 8), dtype=np.float16)
        for c in range(NCALL):
            for g in range(8):
                kl = c * 8 + g
                k = k0 + kl
                if kl < NK and k < D:
                    col = emb2[:, k].astype(np.float16)
                    for j in range(16):
                        tabsb[16 * g + j, c] = col[VC * j:VC * (j + 1)].reshape(NBLK, 2)
                    v = x2c[kl]                           # [512]
                    li = np.arange(L)
                    aidx[16 * g + li % 16, c, li // 16] = ((v % VC) // 2).astype(np.int16)
                    mskw[16 * g + v // VC, c, li, v % 2] = 1
                    # pec[p, c, j, g] = pe[j*128+p, k]
                    pec[:, c, :, g] = pe[:, k].reshape(4, 128).T.astype(np.float16)
        im = {
            "emb1f": emb1f,
            "x1w": x1w,
            "tab": tabsb.reshape(128, -1),
            "aidx": aidx.reshape(128, -1),
            "msk": mskw.reshape(128, -1),
            "g8": g8,
            "pe4": pe4,
            "pec": pec.reshape(128, -1),
            "id8": id8,
        }
        in_maps.append(im)

    res1 = run_bass_kernel_spmd(_phase1(), in_maps,
                                core_ids=list(range(NCORES)), trace=_trace[0])
    diagT = np.zeros((NCORES * NK, D), dtype=np.float32)
    for core, r in enumerate(res1.results):
        dg = r["diag"]                                   # [NCH*8, D]
        for ci, (base, chw) in enumerate(CHUNKS):
            diagT[NK * core + base:NK * core + base + chw] = \
                dg[ci * 8:ci * 8 + chw]
    diagT = diagT[:D]                                     # [300 k, 300 e]

    PW = D + 1 + EC + OUT + 1
    in2_maps = []
    for core in range(NCORES):
        e0 = EC * core
        ne = min(EC, max(0, D - e0))
        pk2 = np.zeros((DP, PW), dtype=np.float32)
        pk2[:D, :D] = w1.T                         # col 300 stays zero
        pk2[:D, D + 1:D + 1 + EC][:, :ne] = diagT[:, e0:e0 + ne]
        pk2[:D, D + 1 + EC:D + 1 + EC + OUT] = w2.T
        pk2[D, D + 1 + EC:D + 1 + EC + OUT] = b2   # b2 rides as w2T row 300
        pk2[:D, PW - 1] = b1
        pk2[D, PW - 1] = 1.0                       # bias makes hT row 300 = 1
        in2_maps.append({"pk2": pk2})
    res2 = run_bass_kernel_spmd(_phase2s(), in2_maps,
                                core_ids=list(range(NCORES)), trace=_trace[1])
    out = np.concatenate([r["out"] for r in res2.results])[:D]

    if _trace[0] or _trace[1]:
        kernel._last_exec_ns = (res1.exec_time_ns, res2.exec_time_ns)
        kernel._last_results = (res1, res2)
    return out


# revision 26
# speedup vs baseline: 1.0326x; 1.0326x over previous
"""Trainium2 Bass kernel for nn_Matposer_51007031608225.

Only the diagonal of the reference's [512,300,300] bmm is needed:

    diagT[k, e] = sum_l a_k[l] * (scale*emb1[x1[k,l],e] + pe[l,e])
    a_k[l]      = scale*emb2[x2[k,l],k] + pe[l,k]

Phase 1 (SPMD x8, k-sharded 38 per core, 5 pipelined chunks of 8 k):
  - emb1 rows gathered as fp16 (elem 384 = 768B, the 256B-granule optimum):
    halves the dominant DMA stream vs fp32.
  - the emb2 a-values come from an SBUF-resident per-core column slice
    (sequential fp16 load, [128 partitions = 16 vocab-chunks x 8 groups])
    via gpsimd.ap_gather: call c group g extracts k=c*8+g's 512 values as
    16 per-partition candidates; a host one-hot mask + a one-matmul
    group-sum reduce picks the right vocab chunk/parity.  This replaces
    the per-pair 256B slab dma_gather (27.7us of descriptor-bound DMA)
    with ~14us of otherwise-idle GPSIMD time.
  - a-vectors are transposed (PE identity-matmul) into the wrapped [128,4]
    lhsT layout matching the emb1 gather's row placement; the pe-term
    matmuls accumulate into the same PSUM rows as the per-k matvecs, so a
    single combined diag slice is stored.
Phase 2 (tiny, e-sharded x8): after the host concatenates/re-slices the
  per-core diag rows, relu(diag @ w1.T + b1) @ w2.T + b2 and softmax.
"""

import numpy as np
from contextlib import ExitStack

import concourse.bass as bass
import concourse.bacc as bacc
import concourse.tile as tile
import concourse.mybir as mybir
from concourse import library_config
from concourse.bass_utils import run_bass_kernel_spmd

F32 = mybir.dt.float32
F16 = mybir.dt.float16
I16 = mybir.dt.int16
U8 = mybir.dt.uint8

D = 300          # d_model
L = 512          # sequence length
V = 32000        # vocab
OUT = 4
NCORES = 8
NK = 38          # k's per core (8*38 = 304 >= 300)
EP1 = 384        # padded emb1 row in fp16 (768B = 3x256B)
NCALL = 5
# (base, width) k-chunks; the last chunk's rows are gathered j-major in four
# sub-gathers so its matvec trails the final DMA quarters (short serial tail)
CHUNKS = [(0, 8), (8, 8), (16, 8), (24, 8), (32, 6)]
NCH = len(CHUNKS)
JSPLIT = 1
VC = V // 16     # vocab entries per partition in the ap_gather table (2000)
NBLK = VC // 2   # d=2 blocks per partition (1000)
SCALE = float(np.sqrt(np.float32(D)))


# ---------------------------------------------------------------- phase 1

def _build_phase1(skip=()):
    nc = bacc.Bacc("TRN2", target_bir_lowering=False, debug=False,
                   num_devices=NCORES, num_swdge_queues=2)

    emb1f_d = nc.dram_tensor("emb1f", [V, EP1], F16, kind="ExternalInput").ap()
    x1w_d = nc.dram_tensor("x1w", [128, NK * 32], I16, kind="ExternalInput").ap()
    tab_d = nc.dram_tensor("tab", [128, NCALL * VC], F16, kind="ExternalInput").ap()
    aidx_d = nc.dram_tensor("aidx", [128, NCALL * 32], I16, kind="ExternalInput").ap()
    msk_d = nc.dram_tensor("msk", [128, NCALL * 2 * L], F16, kind="ExternalInput").ap()
    g8_d = nc.dram_tensor("g8", [128, 8], F16, kind="ExternalInput").ap()
    pe4_d = nc.dram_tensor("pe4", [128, 4 * D], F16, kind="ExternalInput").ap()
    pec_d = nc.dram_tensor("pec", [128, NCALL * 32], F16, kind="ExternalInput").ap()
    id8_d = nc.dram_tensor("id8", [8, 8], F32, kind="ExternalInput").ap()
    diag_d = nc.dram_tensor("diag", [NCH * 8, D], F32, kind="ExternalOutput").ap()

    with tile.TileContext(nc) as tc, ExitStack() as ctx:
        nc.gpsimd.load_library(library_config.ap_gather)
        cpool = ctx.enter_context(tc.tile_pool(name="consts", bufs=1))
        g1pool = ctx.enter_context(tc.tile_pool(name="g1", bufs=1))
        spool = ctx.enter_context(tc.tile_pool(name="small", bufs=1))
        appool = ctx.enter_context(tc.tile_pool(name="apg", bufs=2))
        afpool = ctx.enter_context(tc.tile_pool(name="af", bufs=1))
        pk_ps = ctx.enter_context(tc.tile_pool(name="pk", bufs=2, space="PSUM"))
        gs_ps = ctx.enter_context(tc.tile_pool(name="gs", bufs=2, space="PSUM"))
        tp_ps = ctx.enter_context(tc.tile_pool(name="tp", bufs=2, space="PSUM"))

        # x1w first (gates the gather desc-gen), then ap_gather deps
        x1w = cpool.tile([128, NK * 32], I16)
        nc.sync.dma_start(x1w[:], x1w_d[:])
        tab = cpool.tile([128, NCALL * VC], F16)
        nc.sync.dma_start(tab[:], tab_d[:])
        aidx = cpool.tile([128, NCALL * 32], I16)
        nc.sync.dma_start(aidx[:], aidx_d[:])
        msk = cpool.tile([128, NCALL * 2 * L], F16)
        nc.sync.dma_start(msk[:], msk_d[:])
        g8 = cpool.tile([128, 8], F16)
        nc.sync.dma_start(g8[:], g8_d[:])
        id8 = cpool.tile([8, 8], F32)
        nc.sync.dma_start(id8[:], id8_d[:])
        pe4 = cpool.tile([128, 4 * D], F16)
        nc.sync.dma_start(pe4[:], pe4_d[:])
        pec = cpool.tile([128, NCALL * 32], F16)
        nc.sync.dma_start(pec[:], pec_d[:])

        tabv = tab[:].rearrange("p (c b d) -> p c b d", c=NCALL, d=2)
        stg = spool.tile([8, NCH * D], F32)     # staged diag rows, stored once

        # ---- loop A: all emb1 gathers issued first (desc-gen up front, one
        # buffer per chunk so transfers stream back-to-back on the DMA device)
        g1s = []
        for ci, (base, chw) in enumerate(CHUNKS):
            last = (ci == NCH - 1) and JSPLIT > 1
            g1 = g1pool.tile([128, chw * 4 * EP1], F16, tag=f"g1_{ci}")
            if not last:
                ni = chw * L
                nc.gpsimd.dma_gather(
                    out_ap=g1[:].rearrange("p (c e) -> p c e", e=EP1),
                    in_ap=emb1f_d[:],
                    idxs_ap=x1w[:, base * 32:(base + chw) * 32],
                    num_idxs=ni,
                    num_idxs_reg=ni,
                    elem_size=EP1,
                    single_packet=False,
                    queue_num=ci % 2,
                )
            else:
                ni = chw * 128
                for j in range(JSPLIT):
                    nc.gpsimd.dma_gather(
                        out_ap=g1[:, j * chw * EP1:(j + 1) * chw * EP1]
                            .rearrange("p (c e) -> p c e", e=EP1),
                        in_ap=emb1f_d[:],
                        idxs_ap=x1w[:, base * 32 + j * chw * 8:
                                    base * 32 + (j + 1) * chw * 8],
                        num_idxs=ni,
                        num_idxs_reg=ni,
                        elem_size=EP1,
                        single_packet=False,
                        queue_num=j % 2,
                    )
            g1s.append(g1)

        # ---- loop B: a-value chains for every call (independent of emb1)
        affs, sws = [], []
        for c in range(NCALL):
            raw = appool.tile([128, 2 * L], F16, tag="raw")
            nc.gpsimd.ap_gather(
                out_ap=raw[:].rearrange("p (i d) -> p i d", d=2),
                in_ap=tabv[:, c],
                idxs_ap=aidx[:, c * 32:(c + 1) * 32],
                channels=128, num_elems=NBLK, d=2, num_idxs=L,
            )
            masked = appool.tile([128, 2 * L], F16, tag="mskd")
            nc.vector.tensor_tensor(
                out=masked[:], in0=raw[:],
                in1=msk[:, c * 2 * L:(c + 1) * 2 * L],
                op=mybir.AluOpType.mult)
            asb = appool.tile([8, L], F32, tag="asb")
            for h in range(2):
                ps = gs_ps.tile([8, L], F32, tag="gs")
                nc.tensor.matmul(out=ps[:], lhsT=g8[:],
                                 rhs=masked[:, h * L:(h + 1) * L],
                                 start=True, stop=True)
                psv = ps[:].rearrange("g (i d) -> g i d", d=2)
                HL = L // 2
                tmp = appool.tile([8, HL], F32, tag=f"tmp{h}")
                nc.vector.tensor_copy(tmp[:], psv[:, :, 0])
                nc.vector.tensor_tensor(
                    out=asb[:, h * HL:(h + 1) * HL],
                    in0=tmp[:], in1=psv[:, :, 1],
                    op=mybir.AluOpType.add)
            # transpose [8, 512] -> wrapped [128, (j g)] fp16
            a_full = afpool.tile([128, 32], F16, tag=f"af{c}")
            s_wide = afpool.tile([128, 4 * 64], F16, tag=f"sw{c}")
            swv = s_wide[:].rearrange("p (j g m) -> p j g m", j=4, g=8)
            nc.vector.memset(s_wide[:], 0.0)
            for j in range(4):
                tp = tp_ps.tile([128, 8], F32, tag="tp")
                nc.tensor.transpose(tp[:], asb[:, j * 128:(j + 1) * 128], id8[:])
                nc.scalar.activation(
                    a_full[:, j * 8:j * 8 + 8], tp[:],
                    mybir.ActivationFunctionType.Copy, scale=SCALE)
            nc.vector.tensor_tensor(
                out=a_full[:], in0=a_full[:],
                in1=pec[:, c * 32:(c + 1) * 32],
                op=mybir.AluOpType.add)
            afv = a_full[:].rearrange("p (j g) -> p j g", j=4)
            for j in range(4):
                # scatter scale*a into the block-diagonal lhsT slots
                nc.vector.tensor_scalar_mul(
                    swv[:, j].rearrange("p g m -> p (g m)")[:, ::9],
                    afv[:, j], SCALE)
            affs.append(afv)
            sws.append(swv)

        # ---- loop C: pe-term + per-k matvecs, one PSUM tile per chunk
        for ci, (base, chw) in enumerate(CHUNKS):
            c, g0 = base // 8, base % 8
            afv, swv, g1 = affs[c], sws[c], g1s[ci]
            pk = pk_ps.tile([8, D], F32, tag="pk")
            for j in range(4):
                nc.tensor.matmul(
                    out=pk[0:chw, :],
                    lhsT=afv[:, j, g0:g0 + chw],
                    rhs=pe4[:, j * D:(j + 1) * D],
                    start=(j == 0), stop=False,
                    skip_group_check=True,
                )
            last = (ci == NCH - 1) and JSPLIT > 1
            order = ([(j, gg) for j in range(4) for gg in range(chw)]
                     if last else
                     [(j, gg) for gg in range(chw) for j in range(4)])
            for n, (j, gg) in enumerate(order):
                blk = (j * chw + gg) if last else (gg * 4 + j)
                nc.tensor.matmul(
                    out=pk[0:chw, :],
                    lhsT=swv[:, j, g0 + gg, g0:g0 + chw],
                    rhs=g1[:, blk * EP1:blk * EP1 + D],
                    start=False, stop=(n == len(order) - 1),
                    skip_group_check=True,
                )
            nc.scalar.activation(stg[0:chw, ci * D:(ci + 1) * D], pk[0:chw, :],
                                 mybir.ActivationFunctionType.Copy, scale=1.0)

        nc.sync.dma_start(
            diag_d[:].rearrange("(c g) e -> g c e", c=NCH),
            stg[:].rearrange("g (c e) -> g c e", c=NCH))

    nc.compile()
    return nc


EC = 38   # e-columns of the head computed per core (8*38 = 304 >= 300)
DP = 384


def _build_phase2s():
    """e-sharded head: every core gets the full diag k-rows but only its own
    38-column e-slice; computes [38, 4] output rows.  All inputs arrive in
    one packed [DP, 343] f32 tensor: [w1T | dS | w2T+b2 | b1]; b2 rides as
    w2T's row 300 against a ones-row injected into hT, and the softmax skips
    the max-subtraction (logits are O(50) at most, safe in f32 exp)."""
    PW = D + 1 + EC + OUT + 1    # 344 packed columns (w1T gets a zero col)
    nc = bacc.Bacc("TRN2", target_bir_lowering=False, debug=False,
                   num_devices=NCORES)

    pk_d = nc.dram_tensor("pk2", [DP, PW], F32, kind="ExternalInput").ap()
    out_d = nc.dram_tensor("out", [EC, OUT], F32, kind="ExternalOutput").ap()

    with tile.TileContext(nc) as tc, ExitStack() as ctx:
        pool = ctx.enter_context(tc.tile_pool(name="p2", bufs=1))
        psum = ctx.enter_context(tc.tile_pool(name="ps2", bufs=1, space="PSUM"))

        pkt = pool.tile([128, 3 * PW], F32)
        pkv = pk_d[:].rearrange("(c p) x -> c p x", p=128)
        for i in range(3):
            nc.sync.dma_start(pkt[:, i * PW:(i + 1) * PW], pkv[i])
        w1T = [pkt[:, i * PW:i * PW + D + 1] for i in range(3)]
        dT = [pkt[:, i * PW + D + 1:i * PW + D + 1 + EC] for i in range(3)]
        w2t = [pkt[:, i * PW + D + 1 + EC:i * PW + D + 1 + EC + OUT]
               for i in range(3)]
        b1t = [pkt[:, i * PW + PW - 1:i * PW + PW] for i in range(3)]

        # hT[j, e'] = relu(sum_k w1T[k, j] dT[k, e'] + b1[j]); j=300 is an
        # all-ones row (w1T col 300 = 0, b1[300] = 1) pairing with w2T row
        # 300 = b2, so the b2 bias rides the logits matmul
        JC = [(0, 128), (128, 128), (256, 45)]
        hT = []
        for jm, (j0, jn) in enumerate(JC):
            ph = psum.tile([128, EC], F32, tag=f"ph{jm}", space="PSUM")
            for kc in range(3):
                nc.tensor.matmul(
                    out=ph[:jn, :],
                    lhsT=w1T[kc][:, j0:j0 + jn],
                    rhs=dT[kc],
                    start=(kc == 0), stop=(kc == 2))
            th = pool.tile([128, EC], F32, tag=f"h{jm}")
            nc.scalar.activation(th[:jn, :], ph[:jn, :],
                                 mybir.ActivationFunctionType.Relu,
                                 bias=b1t[jm][:jn, :], scale=1.0)
            hT.append(th)

        # logits[e', o] = sum_j hT[j, e'] w2T[j, o]  (+b2 via ones-row)
        pl = psum.tile([128, OUT], F32, tag="pl", space="PSUM")
        for jm, (j0, jn) in enumerate(JC):
            nc.tensor.matmul(
                out=pl[:EC, :],
                lhsT=hT[jm][:jn, :],
                rhs=w2t[jm][:jn, :],
                start=(jm == 0), stop=(jm == 2))
        nmax = pool.tile([128, 1], F32, tag="nm")
        nc.vector.reduce_max(nmax[:EC, :], pl[:EC, :],
                             axis=mybir.AxisListType.X, negate=True)
        ex = pool.tile([128, OUT], F32, tag="ex")
        ssum = pool.tile([128, 1], F32, tag="ss")
        nc.scalar.activation(ex[:EC, :], pl[:EC, :],
                             mybir.ActivationFunctionType.Exp,
                             bias=nmax[:EC, :], scale=1.0,
                             accum_out=ssum[:EC, :])
        rcp = pool.tile([128, 1], F32, tag="rc")
        nc.vector.reciprocal(rcp[:EC, :], ssum[:EC, :])
        so = pool.tile([128, OUT], F32, tag="so")
        nc.vector.tensor_scalar_mul(so[:EC, :], ex[:EC, :], rcp[:EC, :])
        nc.sync.dma_start(out_d[:], so[:EC, :])

    nc.compile()
    return nc


_CACHE = {}
FUSED = False   # kept for test.py compatibility


def _phase1(fused=False):
    if "p1" not in _CACHE:
        _CACHE["p1"] = _build_phase1()
    return _CACHE["p1"]


def _phase2s():
    if "p2s" not in _CACHE:
        _CACHE["p2s"] = _build_phase2s()
    return _CACHE["p2s"]


# ---------------------------------------------------------------- host glue

def _pe_table():
    pos = np.arange(L, dtype=np.float32)[:, None]
    div = np.exp(np.arange(0, D, 2, dtype=np.float32)
                 * np.float32(-np.log(10000.0) / D))
    pe = np.zeros((L, D), dtype=np.float32)
    pe[:, 0::2] = np.sin(pos * div)
    pe[:, 1::2] = np.cos(pos * div)
    return pe


def _wrap_idx(rows):
    """rows [nk, 512] -> int16 [128, nk*32] in dma_gather's wrapped layout
    (per CHUNKS blocks; idx i of a chunk sits at [i%16, blockcol+i//16],
    replicated down all 128 partitions)."""
    out = np.zeros((16, rows.shape[0] * 32), dtype=np.int16)
    for ci, (base, chw) in enumerate(CHUNKS):
        blk = rows[base:base + chw]                     # [chw, 512]
        if ci == len(CHUNKS) - 1 and JSPLIT > 1:
            # j-major stream: position = j*chw*128 + k*128 + r
            seq = blk.reshape(chw, 4, 128).transpose(1, 0, 2).reshape(-1)
        else:
            seq = blk.reshape(-1)                       # chw*512, l-major
        out[:, base * 32:base * 32 + chw * 32] = seq.reshape(-1, 16).T
    return np.tile(out, (8, 1))


def kernel(x1, x2, emb1, emb2, w1, b1, w2, b2, _trace=(False, False)):
    x1 = np.asarray(x1); x2 = np.asarray(x2)
    emb1 = np.asarray(emb1, dtype=np.float32)
    emb2 = np.asarray(emb2, dtype=np.float32)
    w1 = np.asarray(w1, dtype=np.float32); b1 = np.asarray(b1, dtype=np.float32)
    w2 = np.asarray(w2, dtype=np.float32); b2 = np.asarray(b2, dtype=np.float32)

    pe = _pe_table()
    emb1f = np.zeros((V, EP1), dtype=np.float16)
    emb1f[:, :D] = emb1.astype(np.float16)

    # pe4: [p, j*300+e] = pe[j*128+p, e]
    pe4 = np.ascontiguousarray(
        pe.reshape(4, 128, D).transpose(1, 0, 2).reshape(128, 4 * D)
    ).astype(np.float16)

    g8 = np.zeros((128, 8), dtype=np.float16)
    for g in range(8):
        g8[16 * g:16 * (g + 1), g] = 1.0
    id8 = np.eye(8, dtype=np.float32)

    in_maps = []
    for core in range(NCORES):
        k0 = NK * core
        x1w = _wrap_idx(x1[k0:k0 + NK].astype(np.int64))

        x2c = x2[k0:k0 + NK].astype(np.int64)            # [38, 512]
        tabsb = np.zeros((128, NCALL, NBLK, 2), dtype=np.float16)
        aidx = np.zeros((128, NCALL, 32), dtype=np.int16)
        mskw = np.zeros((128, NCALL, L, 2), dtype=np.float16)
        pec = np.zeros((128, NCALL, 4, 8), dtype=np.float16)
        for c in range(NCALL):
            for g in range(8):
                kl = c * 8 + g
                k = k0 + kl
                if kl < NK and k < D:
                    col = emb2[:, k].astype(np.float16)
                    for j in range(16):
                        tabsb[16 * g + j, c] = col[VC * j:VC * (j + 1)].reshape(NBLK, 2)
                    v = x2c[kl]                           # [512]
                    li = np.arange(L)
                    aidx[16 * g + li % 16, c, li // 16] = ((v % VC) // 2).astype(np.int16)
                    mskw[16 * g + v // VC, c, li, v % 2] = 1.0
                    # pec[p, c, j, g] = pe[j*128+p, k]
                    pec[:, c, :, g] = pe[:, k].reshape(4, 128).T.astype(np.float16)
        im = {
            "emb1f": emb1f,
            "x1w": x1w,
            "tab": tabsb.reshape(128, -1),
            "aidx": aidx.reshape(128, -1),
            "msk": mskw.reshape(128, -1),
            "g8": g8,
            "pe4": pe4,
            "pec": pec.reshape(128, -1),
            "id8": id8,
        }
        in_maps.append(im)

    res1 = run_bass_kernel_spmd(_phase1(), in_maps,
                                core_ids=list(range(NCORES)), trace=_trace[0])
    diagT = np.zeros((NCORES * NK, D), dtype=np.float32)
    for core, r in enumerate(res1.results):
        dg = r["diag"]                                   # [NCH*8, D]
        for ci, (base, chw) in enumerate(CHUNKS):
            diagT[NK * core + base:NK * core + base + chw] = \
                dg[ci * 8:ci * 8 + chw]
    diagT = diagT[:D]                                     # [300 k, 300 e]

    PW = D + 1 + EC + OUT + 1
    in2_maps = []
    for core in range(NCORES):
        e0 = EC * core
        ne = min(EC, max(0, D - e0))
        pk2 = np.zeros((DP, PW), dtype=np.float32)
        pk2[:D, :D] = w1.T                         # col 300 stays zero
        pk2[:D, D + 1:D + 1 + EC][:, :ne] = diagT[:, e0:e0 + ne]
        pk2[:D, D + 1 + EC:D + 1 + EC + OUT] = w2.T
        pk2[D, D + 1 + EC:D + 1 + EC + OUT] = b2   # b2 rides as w2T row 300
        pk2[:D, PW - 1] = b1
        pk2[D, PW - 1] = 1.0                       # bias makes hT row 300 = 1
        in2_maps.append({"pk2": pk2})
    res2 = run_bass_kernel_spmd(_phase2s(), in2_maps,
                                core_ids=list(range(NCORES)), trace=_trace[1])
    out = np.concatenate([r["out"] for r in res2.results])[:D]

    if _trace[0] or _trace[1]:
        kernel._last_exec_ns = (res1.exec_time_ns, res2.exec_time_ns)
        kernel._last_results = (res1, res2)
    return out


# revision 28
# speedup vs baseline: 1.0330x; 1.0004x over previous
"""Trainium2 Bass kernel for nn_Matposer_51007031608225.

Only the diagonal of the reference's [512,300,300] bmm is needed:

    diagT[k, e] = sum_l a_k[l] * (scale*emb1[x1[k,l],e] + pe[l,e])
    a_k[l]      = scale*emb2[x2[k,l],k] + pe[l,k]

Phase 1 (SPMD x8, k-sharded 38 per core, 5 pipelined chunks of 8 k):
  - emb1 rows gathered as fp16 (elem 384 = 768B, the 256B-granule optimum):
    halves the dominant DMA stream vs fp32.
  - the emb2 a-values come from an SBUF-resident per-core column slice
    (sequential fp16 load, [128 partitions = 16 vocab-chunks x 8 groups])
    via gpsimd.ap_gather: call c group g extracts k=c*8+g's 512 values as
    16 per-partition candidates; a host one-hot mask + a one-matmul
    group-sum reduce picks the right vocab chunk/parity.  This replaces
    the per-pair 256B slab dma_gather (27.7us of descriptor-bound DMA)
    with ~14us of otherwise-idle GPSIMD time.
  - a-vectors are transposed (PE identity-matmul) into the wrapped [128,4]
    lhsT layout matching the emb1 gather's row placement; the pe-term
    matmuls accumulate into the same PSUM rows as the per-k matvecs, so a
    single combined diag slice is stored.
Phase 2 (tiny, e-sharded x8): after the host concatenates/re-slices the
  per-core diag rows, relu(diag @ w1.T + b1) @ w2.T + b2 and softmax.
"""

import numpy as np
from contextlib import ExitStack

import concourse.bass as bass
import concourse.bacc as bacc
import concourse.tile as tile
import concourse.mybir as mybir
from concourse import library_config
from concourse.bass_utils import run_bass_kernel_spmd

F32 = mybir.dt.float32
F16 = mybir.dt.float16
I16 = mybir.dt.int16
U8 = mybir.dt.uint8

D = 300          # d_model
L = 512          # sequence length
V = 32000        # vocab
OUT = 4
NCORES = 8
NK = 38          # k's per core (8*38 = 304 >= 300)
EP1 = 384        # padded emb1 row in fp16 (768B = 3x256B)
NCALL = 5
# (base, width) k-chunks; the last chunk's rows are gathered j-major in four
# sub-gathers so its matvec trails the final DMA quarters (short serial tail)
CHUNKS = [(0, 8), (8, 8), (16, 8), (24, 8), (32, 6)]
NCH = len(CHUNKS)
JSPLIT = 1
VC = V // 16     # vocab entries per partition in the ap_gather table (2000)
NBLK = VC // 2   # d=2 blocks per partition (1000)
SCALE = float(np.sqrt(np.float32(D)))


# ---------------------------------------------------------------- phase 1

def _build_phase1(skip=()):
    nc = bacc.Bacc("TRN2", target_bir_lowering=False, debug=False,
                   num_devices=NCORES, num_swdge_queues=2)

    emb1f_d = nc.dram_tensor("emb1f", [V, EP1], F16, kind="ExternalInput").ap()
    x1w_d = nc.dram_tensor("x1w", [128, NK * 32], I16, kind="ExternalInput").ap()
    tab_d = nc.dram_tensor("tab", [128, NCALL * VC], F16, kind="ExternalInput").ap()
    aidx_d = nc.dram_tensor("aidx", [128, NCALL * 32], I16, kind="ExternalInput").ap()
    msk_d = nc.dram_tensor("msk", [128, NCALL * 2 * L], F16, kind="ExternalInput").ap()
    g8_d = nc.dram_tensor("g8", [128, 8], F16, kind="ExternalInput").ap()
    pe4_d = nc.dram_tensor("pe4", [128, 4 * D], F16, kind="ExternalInput").ap()
    pec_d = nc.dram_tensor("pec", [128, NCALL * 32], F16, kind="ExternalInput").ap()
    id8_d = nc.dram_tensor("id8", [8, 8], F32, kind="ExternalInput").ap()
    diag_d = nc.dram_tensor("diag", [NCH * 8, D], F32, kind="ExternalOutput").ap()

    with tile.TileContext(nc) as tc, ExitStack() as ctx:
        nc.gpsimd.load_library(library_config.ap_gather)
        cpool = ctx.enter_context(tc.tile_pool(name="consts", bufs=1))
        g1pool = ctx.enter_context(tc.tile_pool(name="g1", bufs=1))
        spool = ctx.enter_context(tc.tile_pool(name="small", bufs=1))
        appool = ctx.enter_context(tc.tile_pool(name="apg", bufs=2))
        afpool = ctx.enter_context(tc.tile_pool(name="af", bufs=1))
        pk_ps = ctx.enter_context(tc.tile_pool(name="pk", bufs=2, space="PSUM"))
        gs_ps = ctx.enter_context(tc.tile_pool(name="gs", bufs=2, space="PSUM"))
        tp_ps = ctx.enter_context(tc.tile_pool(name="tp", bufs=2, space="PSUM"))

        # x1w first (gates the gather desc-gen), then ap_gather deps
        x1w = cpool.tile([128, NK * 32], I16)
        nc.sync.dma_start(x1w[:], x1w_d[:])
        tab = cpool.tile([128, NCALL * VC], F16)
        nc.sync.dma_start(tab[:], tab_d[:])
        aidx = cpool.tile([128, NCALL * 32], I16)
        nc.sync.dma_start(aidx[:], aidx_d[:])
        msk = cpool.tile([128, NCALL * 2 * L], F16)
        nc.sync.dma_start(msk[:], msk_d[:])
        g8 = cpool.tile([128, 8], F16)
        nc.sync.dma_start(g8[:], g8_d[:])
        id8 = cpool.tile([8, 8], F32)
        nc.sync.dma_start(id8[:], id8_d[:])
        pe4 = cpool.tile([128, 4 * D], F16)
        nc.sync.dma_start(pe4[:], pe4_d[:])
        pec = cpool.tile([128, NCALL * 32], F16)
        nc.sync.dma_start(pec[:], pec_d[:])

        tabv = tab[:].rearrange("p (c b d) -> p c b d", c=NCALL, d=2)
        stg = spool.tile([8, NCH * D], F32)     # staged diag rows, stored once

        # ---- loop A: all emb1 gathers issued first (desc-gen up front, one
        # buffer per chunk so transfers stream back-to-back on the DMA device)
        g1s = []
        for ci, (base, chw) in enumerate(CHUNKS):
            last = (ci == NCH - 1) and JSPLIT > 1
            g1 = g1pool.tile([128, chw * 4 * EP1], F16, tag=f"g1_{ci}")
            if not last:
                ni = chw * L
                nc.gpsimd.dma_gather(
                    out_ap=g1[:].rearrange("p (c e) -> p c e", e=EP1),
                    in_ap=emb1f_d[:],
                    idxs_ap=x1w[:, base * 32:(base + chw) * 32],
                    num_idxs=ni,
                    num_idxs_reg=ni,
                    elem_size=EP1,
                    single_packet=False,
                    queue_num=ci % 2,
                )
            else:
                ni = chw * 128
                for j in range(JSPLIT):
                    nc.gpsimd.dma_gather(
                        out_ap=g1[:, j * chw * EP1:(j + 1) * chw * EP1]
                            .rearrange("p (c e) -> p c e", e=EP1),
                        in_ap=emb1f_d[:],
                        idxs_ap=x1w[:, base * 32 + j * chw * 8:
                                    base * 32 + (j + 1) * chw * 8],
                        num_idxs=ni,
                        num_idxs_reg=ni,
                        elem_size=EP1,
                        single_packet=False,
                        queue_num=j % 2,
                    )
            g1s.append(g1)

        # ---- loop B: a-value chains for every call (independent of emb1)
        affs, sws = [], []
        for c in range(NCALL):
            raw = appool.tile([128, 2 * L], F16, tag="raw")
            nc.gpsimd.ap_gather(
                out_ap=raw[:].rearrange("p (i d) -> p i d", d=2),
                in_ap=tabv[:, c],
                idxs_ap=aidx[:, c * 32:(c + 1) * 32],
                channels=128, num_elems=NBLK, d=2, num_idxs=L,
            )
            masked = appool.tile([128, 2 * L], F16, tag="mskd")
            nc.vector.tensor_tensor(
                out=masked[:], in0=raw[:],
                in1=msk[:, c * 2 * L:(c + 1) * 2 * L],
                op=mybir.AluOpType.mult)
            asb = appool.tile([8, L], F32, tag="asb")
            for h in range(2):
                ps = gs_ps.tile([8, L], F32, tag="gs")
                nc.tensor.matmul(out=ps[:], lhsT=g8[:],
                                 rhs=masked[:, h * L:(h + 1) * L],
                                 start=True, stop=True)
                psv = ps[:].rearrange("g (i d) -> g i d", d=2)
                HL = L // 2
                tmp = appool.tile([8, HL], F32, tag=f"tmp{h}")
                nc.vector.tensor_copy(tmp[:], psv[:, :, 0])
                nc.vector.tensor_tensor(
                    out=asb[:, h * HL:(h + 1) * HL],
                    in0=tmp[:], in1=psv[:, :, 1],
                    op=mybir.AluOpType.add)
            # transpose [8, 512] -> wrapped [128, (j g)] fp16
            a_full = afpool.tile([128, 32], F16, tag=f"af{c}")
            s_wide = afpool.tile([128, 4 * 64], F16, tag=f"sw{c}")
            swv = s_wide[:].rearrange("p (j g m) -> p j g m", j=4, g=8)
            nc.vector.memset(s_wide[:], 0.0)
            for j in range(4):
                tp = tp_ps.tile([128, 8], F32, tag="tp")
                nc.tensor.transpose(tp[:], asb[:, j * 128:(j + 1) * 128], id8[:])
                nc.scalar.activation(
                    a_full[:, j * 8:j * 8 + 8], tp[:],
                    mybir.ActivationFunctionType.Copy, scale=SCALE)
            nc.vector.tensor_tensor(
                out=a_full[:], in0=a_full[:],
                in1=pec[:, c * 32:(c + 1) * 32],
                op=mybir.AluOpType.add)
            afv = a_full[:].rearrange("p (j g) -> p j g", j=4)
            for j in range(4):
                # scatter scale*a into the block-diagonal lhsT slots
                nc.vector.tensor_scalar_mul(
                    swv[:, j].rearrange("p g m -> p (g m)")[:, ::9],
                    afv[:, j], SCALE)
            affs.append(afv)
            sws.append(swv)

        # ---- loop C: pe-term + per-k matvecs, one PSUM tile per chunk
        for ci, (base, chw) in enumerate(CHUNKS):
            c, g0 = base // 8, base % 8
            afv, swv, g1 = affs[c], sws[c], g1s[ci]
            pk = pk_ps.tile([8, D], F32, tag="pk")
            for j in range(4):
                nc.tensor.matmul(
                    out=pk[0:chw, :],
                    lhsT=afv[:, j, g0:g0 + chw],
                    rhs=pe4[:, j * D:(j + 1) * D],
                    start=(j == 0), stop=False,
                    skip_group_check=True,
                )
            last = (ci == NCH - 1) and JSPLIT > 1
            order = ([(j, gg) for j in range(4) for gg in range(chw)]
                     if last else
                     [(j, gg) for gg in range(chw) for j in range(4)])
            for n, (j, gg) in enumerate(order):
                blk = (j * chw + gg) if last else (gg * 4 + j)
                nc.tensor.matmul(
                    out=pk[0:chw, :],
                    lhsT=swv[:, j, g0 + gg, g0:g0 + chw],
                    rhs=g1[:, blk * EP1:blk * EP1 + D],
                    start=False, stop=(n == len(order) - 1),
                    skip_group_check=True,
                )
            nc.scalar.activation(stg[0:chw, ci * D:(ci + 1) * D], pk[0:chw, :],
                                 mybir.ActivationFunctionType.Copy, scale=1.0)

        nc.sync.dma_start(
            diag_d[:].rearrange("(c g) e -> g c e", c=NCH),
            stg[:].rearrange("g (c e) -> g c e", c=NCH))

    nc.compile()
    return nc


EC = 38   # e-columns of the head computed per core (8*38 = 304 >= 300)
DP = 384


def _build_phase2s():
    """e-sharded head: every core gets the full diag k-rows but only its own
    38-column e-slice; computes [38, 4] output rows.  All inputs arrive in
    one packed [DP, 343] f32 tensor: [w1T | dS | w2T+b2 | b1]; b2 rides as
    w2T's row 300 against a ones-row injected into hT, and the softmax skips
    the max-subtraction (logits are O(50) at most, safe in f32 exp)."""
    PW = D + 1 + EC + OUT + 1    # 344 packed columns (w1T gets a zero col)
    nc = bacc.Bacc("TRN2", target_bir_lowering=False, debug=False,
                   num_devices=NCORES)

    pk_d = nc.dram_tensor("pk2", [DP, PW], F32, kind="ExternalInput").ap()
    out_d = nc.dram_tensor("out", [EC, OUT], F32, kind="ExternalOutput").ap()

    with tile.TileContext(nc) as tc, ExitStack() as ctx:
        pool = ctx.enter_context(tc.tile_pool(name="p2", bufs=1))
        psum = ctx.enter_context(tc.tile_pool(name="ps2", bufs=1, space="PSUM"))

        pkt = pool.tile([128, 3 * PW], F32)
        pkv = pk_d[:].rearrange("(c p) x -> c p x", p=128)
        for eng, i in zip((nc.sync, nc.scalar, nc.sync), range(3)):
            eng.dma_start(pkt[:, i * PW:(i + 1) * PW], pkv[i])
        w1T = [pkt[:, i * PW:i * PW + D + 1] for i in range(3)]
        dT = [pkt[:, i * PW + D + 1:i * PW + D + 1 + EC] for i in range(3)]
        w2t = [pkt[:, i * PW + D + 1 + EC:i * PW + D + 1 + EC + OUT]
               for i in range(3)]
        b1t = [pkt[:, i * PW + PW - 1:i * PW + PW] for i in range(3)]

        # hT[j, e'] = relu(sum_k w1T[k, j] dT[k, e'] + b1[j]); j=300 is an
        # all-ones row (w1T col 300 = 0, b1[300] = 1) pairing with w2T row
        # 300 = b2, so the b2 bias rides the logits matmul
        JC = [(0, 128), (128, 128), (256, 45)]
        hT = []
        for jm, (j0, jn) in enumerate(JC):
            ph = psum.tile([128, EC], F32, tag=f"ph{jm}", space="PSUM")
            for kc in range(3):
                nc.tensor.matmul(
                    out=ph[:jn, :],
                    lhsT=w1T[kc][:, j0:j0 + jn],
                    rhs=dT[kc],
                    start=(kc == 0), stop=(kc == 2))
            th = pool.tile([128, EC], F32, tag=f"h{jm}")
            nc.scalar.activation(th[:jn, :], ph[:jn, :],
                                 mybir.ActivationFunctionType.Relu,
                                 bias=b1t[jm][:jn, :], scale=1.0)
            hT.append(th)

        # logits[e', o] = sum_j hT[j, e'] w2T[j, o]  (+b2 via ones-row)
        pl = psum.tile([128, OUT], F32, tag="pl", space="PSUM")
        for jm, (j0, jn) in enumerate(JC):
            nc.tensor.matmul(
                out=pl[:EC, :],
                lhsT=hT[jm][:jn, :],
                rhs=w2t[jm][:jn, :],
                start=(jm == 0), stop=(jm == 2))
        nmax = pool.tile([128, 1], F32, tag="nm")
        nc.vector.reduce_max(nmax[:EC, :], pl[:EC, :],
                             axis=mybir.AxisListType.X, negate=True)
        ex = pool.tile([128, OUT], F32, tag="ex")
        ssum = pool.tile([128, 1], F32, tag="ss")
        nc.scalar.activation(ex[:EC, :], pl[:EC, :],
                             mybir.ActivationFunctionType.Exp,
                             bias=nmax[:EC, :], scale=1.0,
                             accum_out=ssum[:EC, :])
        rcp = pool.tile([128, 1], F32, tag="rc")
        nc.vector.reciprocal(rcp[:EC, :], ssum[:EC, :])
        so = pool.tile([128, OUT], F32, tag="so")
        nc.vector.tensor_scalar_mul(so[:EC, :], ex[:EC, :], rcp[:EC, :])
        nc.sync.dma_start(out_d[:], so[:EC, :])

    nc.compile()
    return nc


_CACHE = {}
FUSED = False   # kept for test.py compatibility


def _phase1(fused=False):
    if "p1" not in _CACHE:
        _CACHE["p1"] = _build_phase1()
    return _CACHE["p1"]


def _phase2s():
    if "p2s" not in _CACHE:
        _CACHE["p2s"] = _build_phase2s()
    return _CACHE["p2s"]


# ---------------------------------------------------------------- host glue

def _pe_table():
    pos = np.arange(L, dtype=np.float32)[:, None]
    div = np.exp(np.arange(0, D, 2, dtype=np.float32)
                 * np.float32(-np.log(10000.0) / D))
    pe = np.zeros((L, D), dtype=np.float32)
    pe[:, 0::2] = np.sin(pos * div)
    pe[:, 1::2] = np.cos(pos * div)
    return pe


def _wrap_idx(rows):
    """rows [nk, 512] -> int16 [128, nk*32] in dma_gather's wrapped layout
    (per CHUNKS blocks; idx i of a chunk sits at [i%16, blockcol+i//16],
    replicated down all 128 partitions)."""
    out = np.zeros((16, rows.shape[0] * 32), dtype=np.int16)
    for ci, (base, chw) in enumerate(CHUNKS):
        blk = rows[base:base + chw]                     # [chw, 512]
        if ci == len(CHUNKS) - 1 and JSPLIT > 1:
            # j-major stream: position = j*chw*128 + k*128 + r
            seq = blk.reshape(chw, 4, 128).transpose(1, 0, 2).reshape(-1)
        else:
            seq = blk.reshape(-1)                       # chw*512, l-major
        out[:, base * 32:base * 32 + chw * 32] = seq.reshape(-1, 16).T
    return np.tile(out, (8, 1))


def kernel(x1, x2, emb1, emb2, w1, b1, w2, b2, _trace=(False, False)):
    x1 = np.asarray(x1); x2 = np.asarray(x2)
    emb1 = np.asarray(emb1, dtype=np.float32)
    emb2 = np.asarray(emb2, dtype=np.float32)
    w1 = np.asarray(w1, dtype=np.float32); b1 = np.asarray(b1, dtype=np.float32)
    w2 = np.asarray(w2, dtype=np.float32); b2 = np.asarray(b2, dtype=np.float32)

    pe = _pe_table()
    emb1f = np.zeros((V, EP1), dtype=np.float16)
    emb1f[:, :D] = emb1.astype(np.float16)

    # pe4: [p, j*300+e] = pe[j*128+p, e]
    pe4 = np.ascontiguousarray(
        pe.reshape(4, 128, D).transpose(1, 0, 2).reshape(128, 4 * D)
    ).astype(np.float16)

    g8 = np.zeros((128, 8), dtype=np.float16)
    for g in range(8):
        g8[16 * g:16 * (g + 1), g] = 1.0
    id8 = np.eye(8, dtype=np.float32)

    in_maps = []
    for core in range(NCORES):
        k0 = NK * core
        x1w = _wrap_idx(x1[k0:k0 + NK].astype(np.int64))

        x2c = x2[k0:k0 + NK].astype(np.int64)            # [38, 512]
        tabsb = np.zeros((128, NCALL, NBLK, 2), dtype=np.float16)
        aidx = np.zeros((128, NCALL, 32), dtype=np.int16)
        mskw = np.zeros((128, NCALL, L, 2), dtype=np.float16)
        pec = np.zeros((128, NCALL, 4, 8), dtype=np.float16)
        for c in range(NCALL):
            for g in range(8):
                kl = c * 8 + g
                k = k0 + kl
                if kl < NK and k < D:
                    col = emb2[:, k].astype(np.float16)
                    for j in range(16):
                        tabsb[16 * g + j, c] = col[VC * j:VC * (j + 1)].reshape(NBLK, 2)
                    v = x2c[kl]                           # [512]
                    li = np.arange(L)
                    aidx[16 * g + li % 16, c, li // 16] = ((v % VC) // 2).astype(np.int16)
                    mskw[16 * g + v // VC, c, li, v % 2] = 1.0
                    # pec[p, c, j, g] = pe[j*128+p, k]
                    pec[:, c, :, g] = pe[:, k].reshape(4, 128).T.astype(np.float16)
        im = {
            "emb1f": emb1f,
            "x1w": x1w,
            "tab": tabsb.reshape(128, -1),
            "aidx": aidx.reshape(128, -1),
            "msk": mskw.reshape(128, -1),
            "g8": g8,
            "pe4": pe4,
            "pec": pec.reshape(128, -1),
            "id8": id8,
        }
        in_maps.append(im)

    res1 = run_bass_kernel_spmd(_phase1(), in_maps,
                                core_ids=list(range(NCORES)), trace=_trace[0])
    diagT = np.zeros((NCORES * NK, D), dtype=np.float32)
    for core, r in enumerate(res1.results):
        dg = r["diag"]                                   # [NCH*8, D]
        for ci, (base, chw) in enumerate(CHUNKS):
            diagT[NK * core + base:NK * core + base + chw] = \
                dg[ci * 8:ci * 8 + chw]
    diagT = diagT[:D]                                     # [300 k, 300 e]

    PW = D + 1 + EC + OUT + 1
    in2_maps = []
    for core in range(NCORES):
        e0 = EC * core
        ne = min(EC, max(0, D - e0))
        pk2 = np.zeros((DP, PW), dtype=np.float32)
        pk2[:D, :D] = w1.T                         # col 300 stays zero
        pk2[:D, D + 1:D + 1 + EC][:, :ne] = diagT[:, e0:e0 + ne]
        pk2[:D, D + 1 + EC:D + 1 + EC + OUT] = w2.T
        pk2[D, D + 1 + EC:D + 1 + EC + OUT] = b2   # b2 rides as w2T row 300
        pk2[:D, PW - 1] = b1
        pk2[D, PW - 1] = 1.0                       # bias makes hT row 300 = 1
        in2_maps.append({"pk2": pk2})
    res2 = run_bass_kernel_spmd(_phase2s(), in2_maps,
                                core_ids=list(range(NCORES)), trace=_trace[1])
    out = np.concatenate([r["out"] for r in res2.results])[:D]

    if _trace[0] or _trace[1]:
        kernel._last_exec_ns = (res1.exec_time_ns, res2.exec_time_ns)
        kernel._last_results = (res1, res2)
    return out
